# revision 1
# baseline (speedup 1.0000x reference)
"""Trainium2 Bass kernel for nn_Attention: GPT-2 style attention block.

Per-core work (data-parallel over batch, 1 of 8 batch elements per core):
  qkv = x @ wa + ba ; split q,k,v into 12 heads of 64
  S_h = q_h @ k_h^T            (no 1/sqrt(D) scaling)
  S masked multiplicatively with tril (masked entries ~= 0, still in softmax!)
  P = softmax(S) ; a_h = P @ v_h
  merged[t, d*12+h] = a_h[t, d] ; y = merged @ wp + bp

Key implementation ideas:
  - Big matmuls in float32r (full fp32 bits, fast PE path for N>=256).
  - Scores computed twice: once in [i,j] orientation (row stats only), once in
    [j,i] orientation (for the AV matmul with P^T as moving operand) with
    -(rowmax + lnZ) folded in via K=1 accumulate matmuls, so the exp output is
    already normalized (no per-row division anywhere).
  - Masked-position weights e^{-L_i}: in-diagonal-block wedge fixed with
    copy_predicated from a broadcast tile; fully-masked column blocks feed the
    AV matmul directly from that broadcast tile (scores never materialized).
  - Masked-count contribution to Z enters as one extra ln(count) column in the
    stats reduce + exp-accumulate pass.
  - wp rows permuted at load time to absorb the interleaved (D,H) merge; AV
    head pairs col-packed with tile_position so PSUM is directly the merged
    layout chunk.
  - P^T, v, merged, wp2 in bf16 (values O(1); final rel-err budget allows it).
"""

import math
import sys

sys.path.insert(0, "/opt/trn_rl_repo")

import numpy as np

import concourse.bass as bass
from concourse import bacc
import concourse.mybir as mybir
import concourse.tile as tile
from concourse import bass_utils
from concourse.masks import make_identity

F32 = mybir.dt.float32
F32R = mybir.dt.float32r
BF16 = mybir.dt.bfloat16
AF = mybir.ActivationFunctionType
ALU = mybir.AluOpType

T = 1024
C = 768
H = 12
D = 64
NT = T // 128        # 8 T-chunks
NCC = C // 128       # 6 C-chunks
# PT storage: per head, block b holds score cols [128*b, 1024), width 1024-128b
PT_W = [T - 128 * b for b in range(NT)]
PT_OFF = [sum(PT_W[:b]) for b in range(NT)]
PT_TOT = sum(PT_W)   # 4608


def r32(ap):
    return ap.bitcast(F32R)


def _patch_act_tables():
    from concourse import bacc as _bacc_mod
    import concourse.hw_specs as _hw
    if getattr(_bacc_mod, "_act_tables_patched", False):
        return
    orig = _bacc_mod.get_activation_tables

    def one_set(arch):
        t = orig(arch)
        keep = "natural_log_exp_and_others"
        if keep in t:
            t = {k: (v if k == keep else set()) for k, v in t.items()}
        return t

    _bacc_mod.get_activation_tables = one_set
    _bacc_mod._act_tables_patched = True


def build_nc():
    _patch_act_tables()
    nc = bacc.Bacc("TRN2", target_bir_lowering=False, debug=False, num_devices=8)

    x = nc.dram_tensor("x", [T, C], F32, kind="ExternalInput").ap()
    wa = nc.dram_tensor("wa", [C, 3 * C], F32, kind="ExternalInput").ap()
    ba = nc.dram_tensor("ba", [3 * C], F32, kind="ExternalInput").ap()
    wp = nc.dram_tensor("wp", [C, C], F32, kind="ExternalInput").ap()
    bp = nc.dram_tensor("bp", [C], F32, kind="ExternalInput").ap()
    y = nc.dram_tensor("y", [T, C], F32, kind="ExternalOutput").ap()

    with tile.TileContext(nc) as tc:
        build_attention(tc, x, wa, ba, wp, bp, y)
    nc.compile()
    return nc


def build_attention(tc, x, wa, ba, wp, bp, y):
    nc = tc.nc

    with (
        tc.tile_pool(name="consts", bufs=1) as consts,
        tc.tile_pool(name="persist", bufs=1) as persist,
        tc.tile_pool(name="rows", bufs=1) as rows,
    ):
        # ---------------- constants ----------------
        masks = consts.tile([128, 384], F32, tag="masks")
        ident = masks[:, 0:128]
        tril = masks[:, 128:256]
        make_identity(nc, ident)
        # tril[p, f] = 1 where f <= p (keep), else 0   ([i-part, j-free])
        nc.gpsimd.memset(tril, 1.0)
        nc.gpsimd.affine_select(
            out=tril, in_=tril, compare_op=ALU.is_ge, fill=0.0,
            base=0, pattern=[[-1, 128]], channel_multiplier=1,
        )
        # wedge[p, f] = 1 where p > f  ([j-part, i-free]: masked region j > i)
        wedge = consts.tile([128, 128], mybir.dt.int8, tag="wedge")
        nc.gpsimd.memset(wedge, 1)
        nc.gpsimd.affine_select(
            out=wedge, in_=wedge, compare_op=ALU.is_gt, fill=0,
            base=0, pattern=[[-1, 128]], channel_multiplier=1,
        )
        onesf = consts.tile([2, 128], F32, tag="onesf")
        nc.vector.memset(onesf, 1.0)
        ones2 = consts.tile([2, 128], F32R, tag="ones2")
        nc.scalar.copy(ones2, onesf)
        onesrow = ones2[0:1, :]
        onesb = consts.tile([1, 128], BF16, tag="onesb")
        nc.vector.memset(onesb, 1.0)
        # ln of half the masked-count beyond computed region, per row-chunk r
        # (two columns per r so the fp32r matmul has N=2)
        lncf = consts.tile([1, 2 * NT], F32, tag="lncf")
        nc.vector.memset(lncf, 0.0)
        for r in range(NT - 1):
            nc.vector.memset(
                lncf[:, 2 * r : 2 * r + 2], math.log((T - 128 * (r + 1)) / 2)
            )
        lnc = consts.tile([1, 2 * NT], F32R, tag="lnc")
        nc.scalar.copy(lnc, lncf)
        # bav in [0:768], bp in [768:1536]
        bavbp = consts.tile([1, 2 * C], F32R, tag="bavbp")
        nc.scalar.dma_start(
            out=bavbp[:, 0:C], in_=ba[2 * C : 3 * C].rearrange("(a c) -> a c", a=1).bitcast(F32R)
        )
        nc.scalar.dma_start(
            out=bavbp[:, C : 2 * C], in_=bp.rearrange("(a c) -> a c", a=1).bitcast(F32R)
        )
        bpb = consts.tile([1, C], BF16, tag="bpb")
        nc.scalar.copy(bpb, bavbp[:, C : 2 * C].bitcast(F32))
        # per-partition bias for q/k projection copies: col m = ba[128m:128(m+1)]
        ba_qk = consts.tile([128, 2 * NCC], F32, tag="ba_qk")
        nc.scalar.dma_start(
            out=ba_qk, in_=ba[0 : 2 * C].rearrange("(m p) -> p m", p=128)
        )

        # ---------------- persistent activations ----------------
        qkT = persist.tile([128, 2 * NCC, T], F32R, tag="qkT")  # chunks: 0-5 q, 6-11 k
        v_sb = persist.tile([128, NT, C], BF16, tag="v_sb")    # [t-part, tchunk, feat]
        negLst = persist.tile([128, 8 * H], F32, tag="negLst")  # col 8h+r

        # ---------------- phases: load/transpose; proj+stats interleaved; attn --
        stall = persist.tile([128, 6 * 48], F32, tag="stall")
        wp2 = persist.tile([128, NCC, C], F32R, tag="wp2")
        v_suf = persist.tile([128, 7, C], BF16, tag="v_suf")
        with (
            tc.tile_pool(name="stpsum", bufs=2, space="PSUM") as stpsum,
            tc.tile_pool(name="stsb", bufs=2) as stsb,
        ):
            def emit_stats(pair):
                stpair = stall[:, 48 * pair : 48 * pair + 48]
                for half in range(2):
                    h = 2 * pair + half
                    qm, qp = h // 2, (h % 2) * 64
                    negm = stpair[:, 8 * half : 8 * half + 8]
                    zst = stpair[:, 16 + 8 * half : 24 + 8 * half]
                    lnz = stpair[:, 32 + 8 * half : 40 + 8 * half]
                    for r in range(NT):
                        jcols = 128 * (r + 1)
                        sps = stpsum.tile([128, 1024], F32, tag="ps_s")
                        pieces = [(0, min(jcols, 512))]
                        if jcols > 512:
                            pieces.append((512, jcols - 512))
                        for (p0, pw) in pieces:
                            nc.tensor.matmul(
                                sps[:, p0 : p0 + pw],
                                qkT[qp : qp + 64, qm, 128 * r : 128 * r + 128],
                                qkT[qp : qp + 64, 6 + qm, p0 : p0 + pw],
                                start=True,
                                stop=True,
                            )
                        ncols = jcols
                        if r < NT - 1:
                            # 2 extra cols = ln(count/2) -> exp adds count*e^{-m}
                            nc.tensor.matmul(
                                sps[:, jcols : jcols + 2],
                                onesrow,
                                lnc[:, 2 * r : 2 * r + 2],
                                start=True,
                                stop=True,
                            )
                            ncols = jcols + 2
                        # causal mask on the diagonal 128x128 block
                        nc.vector.tensor_mul(
                            sps[:, 128 * r : 128 * r + 128],
                            sps[:, 128 * r : 128 * r + 128],
                            tril,
                        )
                        nc.vector.reduce_max(
                            negm[:, r : r + 1], sps[:, 0:ncols],
                            axis=mybir.AxisListType.X, negate=True,
                        )
                        scratch = stsb.tile([128, 1026], BF16, tag="scratch")
                        nc.scalar.activation(
                            scratch[:, 0:ncols], sps[:, 0:ncols], AF.Exp,
                            bias=negm[:, r : r + 1],
                            accum_out=zst[:, r : r + 1],
                        )
                    nc.scalar.activation(lnz, zst, AF.Ln)
                    nc.vector.tensor_sub(
                        stpair[:, 8 * half : 8 * half + 8], negm, lnz
                    )
                return stpair

            with (
                tc.tile_pool(name="xload", bufs=1) as xload,
                tc.tile_pool(name="xstream", bufs=4) as xstream,
                tc.tile_pool(name="ph1psum", bufs=2, space="PSUM") as ph1psum,
            ):
                wa_sb = xload.tile([128, NCC, 3 * C], F32R, tag="wa_sb")
                xT = xload.tile([128, NCC, T], F32R, tag="xT")

                def emit_xt(trange):
                    for t in trange:
                        xc = xstream.tile([128, C], F32, tag="xchunk")
                        nc.sync.dma_start(
                            out=xc, in_=x[128 * t : 128 * t + 128, :]
                        )
                        for g in range(2):
                            ps = ph1psum.tile([128, 384], F32, tag="ps_misc")
                            for q in range(3):
                                cc = 3 * g + q
                                nc.tensor.transpose(
                                    ps[:, 128 * q : 128 * q + 128],
                                    xc[:, 128 * cc : 128 * cc + 128], ident,
                                )
                            nc.vector.tensor_copy(
                                xT[:, 3 * g : 3 * g + 3, 128 * t : 128 * t + 128],
                                ps,
                            )

                def emit_projqk(p, ns=(0, 1)):
                    for m in (p, 6 + p):
                        for n in ns:
                            ps = ph1psum.tile([128, 512], F32, tag="ps_proj")
                            for cc in range(NCC):
                                nc.tensor.matmul(
                                    ps,
                                    wa_sb[:, cc, 128 * m : 128 * m + 128],
                                    xT[:, cc, 512 * n : 512 * n + 512],
                                    start=(cc == 0),
                                    stop=(cc == NCC - 1),
                                )
                            nc.scalar.activation(
                                qkT[:, m, 512 * n : 512 * n + 512], ps,
                                AF.Identity, bias=ba_qk[:, m : m + 1],
                            )

                def emit_vproj():
                    for t in range(NT):
                        for n in range(2):
                            ps = ph1psum.tile([128, 384], F32, tag="ps_misc")
                            for cc in range(NCC):
                                nc.tensor.matmul(
                                    ps,
                                    xT[:, cc, 128 * t : 128 * t + 128],
                                    wa_sb[:, cc,
                                          2 * C + 384 * n : 2 * C + 384 * n + 384],
                                    start=(cc == 0),
                                    stop=False,
                                )
                            nc.tensor.matmul(
                                ps,
                                onesrow,
                                bavbp[:, 384 * n : 384 * n + 384],
                                start=False,
                                stop=True,
                            )
                            nc.vector.tensor_copy(
                                v_sb[:, t, 384 * n : 384 * n + 384], ps
                            )

                emit_xt(range(2))
                for cc in range(2):
                    nc.scalar.dma_start(
                        out=wa_sb[:, cc, :],
                        in_=wa[128 * cc : 128 * cc + 128, :].bitcast(F32R),
                    )
                emit_xt(range(2, 4))
                for cc in range(2, NCC):
                    nc.scalar.dma_start(
                        out=wa_sb[:, cc, :],
                        in_=wa[128 * cc : 128 * cc + 128, :].bitcast(F32R),
                    )
                for p in range(6):
                    emit_projqk(p, ns=(0,))
                emit_xt(range(4, NT))
                emit_projqk(0, ns=(1,))
                emit_stats(0)
                emit_projqk(1, ns=(1,))
                emit_stats(1)
                emit_vproj()
                # v_suf[m] = sum of v blocks b > m (for masked-region AV)
                nc.vector.tensor_copy(v_suf[:, 6, :], v_sb[:, 7, :])
                for m in range(5, -1, -1):
                    nc.vector.tensor_add(
                        v_suf[:, m, :], v_suf[:, m + 1, :], v_sb[:, m + 1, :]
                    )
                for p in range(2, 6):
                    emit_projqk(p, ns=(1,))

            # -------- attn phase: per-pair rows -> P^T -> AV -------------------
            with (
                tc.tile_pool(name="rowp", bufs=2) as rowp,
                tc.tile_pool(name="rowh", bufs=2) as rowh,
                tc.tile_pool(name="ptpool", bufs=2) as ptpool,
                tc.tile_pool(name="bexpool", bufs=1) as bexpool,
                tc.tile_pool(name="avpsum", bufs=2, space="PSUM") as avpsum,
                tc.tile_pool(name="stps2", bufs=2, space="PSUM") as stps2,
                tc.tile_pool(name="ph23", bufs=1) as ph23,
                tc.tile_pool(name="ysb", bufs=2) as ysb,
            ):
                mergedT = ph23.tile([128, NCC, T], F32R, tag="mergedT")
                # wp2 load (row-permuted: merged col c2=h*64+d <-> wp row d*12+h)
                wp_r = wp.rearrange("(d h) c -> d h c", h=H)  # [64, 12, 768]
                for k in range(NCC):
                    wst = ysb.tile([128, C], F32, tag="y_stage")
                    for par in range(2):
                        nc.sync.dma_start(
                            out=wst[64 * par : 64 * par + 64, :],
                            in_=wp_r[:, 2 * k + par, :],
                        )
                    nc.scalar.copy(wp2[:, k, :], wst)

                def emit_rows(pair, stpair):
                    rowf = rowp.tile([16, 384], F32, tag="rowf")
                    negLp = rowf[:, 0:128]
                    negLphi = rowf[:, 128:256].bitcast(F32R)
                    negLplo = rowf[:, 256:384].bitcast(F32R)
                    expLp = rowp.tile([16, 128], BF16, tag="expLp")
                    erowp = rowp.tile([1, 2 * T], BF16, tag="erowp")
                    pst = stps2.tile([128, 512], F32, tag="ps_st")
                    nc.tensor.transpose(pst[0:48, 0:128], stpair, ident)
                    nc.scalar.copy(negLp, pst[0:16, 0:128])
                    nc.scalar.copy(negLphi, negLp)
                    nc.vector.tensor_sub(negLplo, negLp, negLphi.bitcast(F32))
                    nc.scalar.activation(expLp, negLp, AF.Exp)
                    nc.sync.dma_start(
                        out=erowp.rearrange("a (p f) -> a p f", p=16), in_=expLp
                    )
                    return negLphi, negLplo, erowp

                def emit_attn(pair, rowsinfo):
                    negLphi, negLplo, erowp = rowsinfo
                    pts = []
                    bexps = []
                    for half in range(2):
                        h = 2 * pair + half
                        qm, qp = h // 2, (h % 2) * 64
                        nlr = rowh.tile([2, T], F32R, tag="nlr")
                        nc.sync.dma_start(
                            out=nlr[0:1, :].rearrange("a (p f) -> a p f", p=8),
                            in_=negLphi[8 * half : 8 * half + 8, :],
                        )
                        nc.sync.dma_start(
                            out=nlr[1:2, :].rearrange("a (p f) -> a p f", p=8),
                            in_=negLplo[8 * half : 8 * half + 8, :],
                        )
                        bexp = bexpool.tile([128, T], BF16, tag=f"bexp{half}")
                        nc.gpsimd.partition_broadcast(
                            bexp, erowp[:, T * half : T * half + T], channels=128
                        )
                        pt = ptpool.tile([128, PT_TOT], BF16, tag=f"pt{half}")
                        for b in range(NT):
                            if b < 4:
                                pieces = [(128 * b, 512 - 128 * b), (512, 512)]
                            else:
                                pieces = [(128 * b, T - 128 * b)]
                            for (g0, gw) in pieces:
                                ps = stps2.tile([128, 512], F32, tag="ps_st")
                                nc.tensor.matmul(
                                    ps[:, 0:gw],
                                    qkT[qp : qp + 64, 6 + qm, 128 * b : 128 * b + 128],
                                    qkT[qp : qp + 64, qm, g0 : g0 + gw],
                                    start=True,
                                    stop=False,
                                )
                                # fold in -(max + lnZ) along the free (i) axis
                                nc.tensor.matmul(
                                    ps[:, 0:gw],
                                    ones2,
                                    nlr[:, g0 : g0 + gw],
                                    start=False,
                                    stop=True,
                                )
                                lo = PT_OFF[b] + g0 - 128 * b
                                nc.scalar.activation(
                                    pt[:, lo : lo + gw], ps[:, 0:gw], AF.Exp
                                )
                            # wedge of diag block -> e^{-L_i}
                            nc.vector.copy_predicated(
                                pt[:, PT_OFF[b] : PT_OFF[b] + 128],
                                wedge,
                                bexp[:, 128 * b : 128 * b + 128],
                            )
                        pts.append(pt)
                        bexps.append(bexp)

                    # AV: out chunk = [headA d (part 0-63) | headB d (part 64-127)]
                    # valid region from PT; masked region: for col range
                    # [128m, 128m+128) all blocks b>m contribute e^{-L_i} * v_b,
                    # i.e. one v_suf[m]^T @ Bexp matmul per range.
                    for c in range(2):
                        ps = avpsum.tile([128, 512], F32, tag="ps_av")
                        for half in range(2):
                            h = 2 * pair + half
                            mms = []
                            for b in range(NT):
                                lo_blk = 128 * b
                                c0, c1 = 512 * c, 512 * c + 512
                                if lo_blk >= c1:
                                    continue
                                g0 = max(lo_blk, c0)
                                lo = PT_OFF[b] + g0 - lo_blk
                                mms.append(
                                    (v_sb[:, b, 64 * h : 64 * h + 64],
                                     pts[half][:, lo : lo + (c1 - g0)], g0 - c0)
                                )
                            for m in range(4 * c, min(4 * c + 4, 7)):
                                mms.append(
                                    (v_suf[:, m, 64 * h : 64 * h + 64],
                                     bexps[half][:, 128 * m : 128 * m + 128],
                                     128 * m - 512 * c)
                                )
                            for idx, (lhsT, rhs, off) in enumerate(mms):
                                nw = rhs.shape[-1]
                                nc.tensor.matmul(
                                    ps[64 * half : 64 * half + 64, off : off + nw],
                                    lhsT, rhs,
                                    start=(idx == 0),
                                    stop=(idx == len(mms) - 1),
                                    tile_position=(0, 64 * half),
                                    skip_group_check=True,
                                )
                        nc.vector.tensor_copy(
                            mergedT[:, pair, 512 * c : 512 * c + 512], ps
                        )

                rinfo = {0: emit_rows(0, stall[:, 0:48])}
                for p in range(6):
                    if p + 2 < 6:
                        emit_stats(p + 2)
                    if p + 1 < 6:
                        rinfo[p + 1] = emit_rows(
                            p + 1, stall[:, 48 * (p + 1) : 48 * (p + 1) + 48]
                        )
                    emit_attn(p, rinfo.pop(p))

                # ---------------- phase 3: c_proj --------------------------------
                for t in range(NT):
                    yt = ysb.tile([128, C], F32, tag="y_stage")
                    for (n0, nw) in ((0, 512), (512, 256)):
                        ps = avpsum.tile([128, 512], F32, tag="ps_av")
                        for k in range(NCC):
                            nc.tensor.matmul(
                                ps[:, 0:nw],
                                mergedT[:, k, 128 * t : 128 * t + 128],
                                wp2[:, k, n0 : n0 + nw],
                                start=(k == 0),
                                stop=False,
                            )
                        nc.tensor.matmul(
                            ps[:, 0:nw],
                            onesrow,
                            bavbp[:, C + n0 : C + n0 + nw],
                            start=False,
                            stop=True,
                        )
                        nc.vector.tensor_copy(yt[:, n0 : n0 + nw], ps[:, 0:nw])
                    nc.sync.dma_start(out=y[128 * t : 128 * t + 128, :], in_=yt)


_NC_CACHE = None


def get_nc():
    global _NC_CACHE
    if _NC_CACHE is None:
        _NC_CACHE = build_nc()
    return _NC_CACHE


def kernel(x, wa, ba, wp, bp, **kw):
    x = np.asarray(x, dtype=np.float32)
    in_maps = [
        {
            "x": np.ascontiguousarray(x[b]),
            "wa": np.asarray(wa, dtype=np.float32),
            "ba": np.asarray(ba, dtype=np.float32),
            "wp": np.asarray(wp, dtype=np.float32),
            "bp": np.asarray(bp, dtype=np.float32),
        }
        for b in range(8)
    ]
    res = bass_utils.run_bass_kernel_spmd(get_nc(), in_maps, core_ids=list(range(8)))
    return np.stack([r["y"] for r in res.results], axis=0)


if __name__ == "__main__":
    nc = build_nc()
    print("build OK, instructions:", sum(1 for _ in nc.m.functions[0].body) if hasattr(nc.m.functions[0], "body") else "n/a")

